# revision 5
# baseline (speedup 1.0000x reference)
"""Trainium2 Bass kernel for nn_ArgumentGCN (2-step GCN message passing).

Math (per batch b, per step):
    d_w[j]   = sigmoid(node[j,:] @ w_nw.T + b_nw)
    self[i,:]= node[i,:] @ w_self.T + b_self
    agg[i,:] = sum_k sum_j g_k[i,j] * d_w[j] * (node @ w_k.T)[j,:]   (g_k in {0,1})
    node     = relu(self + agg / nn_num[:,None])
with g_k[i,j] = mask[i]*mask[j]*offdiag[i,j]*adj_k[i,j] and
nn_num[i] = max(sum_kj g_k[i,j], 1).

Key identities used on device:
  - where(g>0, d_w, 0) @ info == adj_hole @ diag(mask*d_w) @ info, so the
    masked bmm becomes a plain matmul with mask_j folded into the S operand
    and mask_i applied on the output side.
  - nn_num row-sums come for free as one extra bf16 mask-column appended to
    the S operand of the aggregation matmul.
Sharding: pure batch data-parallel, 4 batches per core, no collectives.
Layouts: host pre-transposes adjacency ([b,k,j,i]) and node ([b,d,j]) and
casts to bf16 (adjacency entries are 0/1, exact in bf16) so every DMA is
contiguous; diagonal zeroing (offdiag) happens on device via affine_select.
"""

import numpy as np
import ml_dtypes

import concourse.bass as bass
import concourse.mybir as mybir
import concourse.tile as tile
from concourse import bacc
from concourse.bass_utils import run_bass_kernel_spmd

BF16 = ml_dtypes.bfloat16

B, N, D = 32, 512, 256
STEPS = 2
NCORES = 8
BL = B // NCORES          # batches per core
P = 128
NJC = N // P              # 4 node chunks of 128
KC = D // P               # 2 contraction chunks of 128
NK = 4                    # number of adjacency matrices
TCH = NK * NJC            # 16 stacked chunks in the aggregation contraction

f32 = mybir.dt.float32
bf16 = mybir.dt.bfloat16
AF = mybir.ActivationFunctionType


def build_nc(bl=BL):
    nc = bacc.Bacc(None)

    adjT = nc.declare_dram_parameter("adjT", [bl, NK, N, N], bf16, isOutput=False)
    nodeT = nc.declare_dram_parameter("nodeT", [bl, D, N], bf16, isOutput=False)
    maskf = nc.declare_dram_parameter("maskf", [bl, N], f32, isOutput=False)
    w12 = nc.declare_dram_parameter("w12", [D, 2 * D], bf16, isOutput=False)
    w34 = nc.declare_dram_parameter("w34", [D, 2 * D], bf16, isOutput=False)
    w5 = nc.declare_dram_parameter("w5", [D, D + 1], bf16, isOutput=False)
    brow = nc.declare_dram_parameter("brow", [1, D + 1], bf16, isOutput=False)
    ident = nc.declare_dram_parameter("ident", [P, P], bf16, isOutput=False)

    out_node = nc.declare_dram_parameter("out_node", [bl, N, D], f32, isOutput=True)
    out_w = nc.declare_dram_parameter("out_w", [bl, STEPS, N], f32, isOutput=True)

    with tile.TileContext(nc) as tc:
        with (
            tc.tile_pool(name="const", bufs=1) as const,
            tc.tile_pool(name="gt", bufs=2) as gpool,
            tc.tile_pool(name="nt", bufs=3) as ntpool,
            tc.tile_pool(name="st", bufs=2) as stpool,
            tc.tile_pool(name="sb", bufs=2) as sbpool,
            tc.tile_pool(name="cols", bufs=4) as colpool,
            tc.tile_pool(name="cmb", bufs=3) as cmbpool,
            tc.tile_pool(name="psA", bufs=2, space="PSUM") as psA,
            tc.tile_pool(name="psB", bufs=2, space="PSUM") as psB,
            tc.tile_pool(name="psC", bufs=2, space="PSUM") as psC,
        ):
            # ---- constants ----
            w12t = const.tile([P, KC, 2 * D], bf16, tag="w12")
            nc.sync.dma_start(w12t[:], w12.rearrange("(kc p) f -> p kc f", p=P))
            w34t = const.tile([P, KC, 2 * D], bf16, tag="w34")
            nc.sync.dma_start(w34t[:], w34.rearrange("(kc p) f -> p kc f", p=P))
            w5t = const.tile([P, KC, D + 1], bf16, tag="w5")
            nc.sync.dma_start(w5t[:], w5.rearrange("(kc p) f -> p kc f", p=P))
            browt = const.tile([1, D + 1], bf16, tag="brow")
            nc.sync.dma_start(browt[:], brow[:])
            identt = const.tile([P, P], bf16, tag="ident")
            nc.sync.dma_start(identt[:], ident[:])
            ones1 = const.tile([1, P], bf16, tag="ones")
            nc.vector.memset(ones1[:], 1.0)

            for b in range(bl):
                # ---- per-batch loads ----
                # gt chunk index t = k*NJC + jc ; each chunk is [128 j, 512 i]
                gt = gpool.tile([P, TCH, N], bf16, tag="gt")
                gtv = gt.rearrange("p (k c) n -> p k c n", k=NK)
                for k in range(NK):
                    nc.sync.dma_start(
                        gtv[:, k],
                        adjT[b, k].rearrange("(c p) i -> p c i", p=P),
                    )
                # zero the diagonal: chunk (k, c) has its diagonal at
                # column i = c*128 + x ; keep where (x - i + 128c) != 0
                for k in range(NK):
                    for c in range(NJC):
                        nc.gpsimd.affine_select(
                            out=gtv[:, k, c],
                            in_=gtv[:, k, c],
                            compare_op=mybir.AluOpType.not_equal,
                            fill=0.0,
                            base=P * c,
                            channel_multiplier=1,
                            pattern=[[-1, N]],
                        )

                nt = ntpool.tile([P, KC, N], bf16, tag="nt")
                nc.sync.dma_start(nt[:], nodeT[b].rearrange("(kc p) n -> p kc n", p=P))

                mcol = colpool.tile([P, NJC], f32, tag="mcol")
                with nc.allow_non_contiguous_dma(reason="2KB mask column load"):
                    nc.sync.dma_start(mcol[:], maskf[b].rearrange("(c p) -> p c", p=P))
                mcolb = colpool.tile([P, NJC], bf16, tag="mcolb")
                nc.vector.tensor_copy(mcolb[:], mcol[:])

                fincol = colpool.tile([P, NJC], f32, tag="fin")
                dwt = sbpool.tile([P, STEPS * NJC], f32, tag="dwt")

                for s in range(STEPS):
                    St = stpool.tile([P, TCH, D + 1], bf16, tag="St")
                    Sv = St.rearrange("p (k c) d -> p k c d", k=NK)
                    selfsb = sbpool.tile([P, NJC, D], f32, tag="selfsb")

                    # ---- small matmuls: info_k, self_info, d_w logits ----
                    for jc in range(NJC):
                        ps12 = psA.tile([P, 2 * D], f32, tag="ps12")
                        ps34 = psA.tile([P, 2 * D], f32, tag="ps34")
                        ps5 = psB.tile([P, D + 1], f32, tag="ps5")
                        for kc in range(KC):
                            lhs = nt[:, kc, jc * P:(jc + 1) * P]
                            nc.tensor.matmul(
                                ps12[:], lhsT=lhs, rhs=w12t[:, kc],
                                start=(kc == 0), stop=(kc == KC - 1),
                            )
                            nc.tensor.matmul(
                                ps34[:], lhsT=lhs, rhs=w34t[:, kc],
                                start=(kc == 0), stop=(kc == KC - 1),
                            )
                            nc.tensor.matmul(
                                ps5[:], lhsT=lhs, rhs=w5t[:, kc],
                                start=(kc == 0), stop=False,
                            )
                        # bias row: adds [b_self | b_nw] via K=1 matmul
                        nc.tensor.matmul(
                            ps5[:], lhsT=ones1[:], rhs=browt[:],
                            start=False, stop=True,
                        )

                        # ---- evictions ----
                        dwslot = dwt[:, s * NJC + jc: s * NJC + jc + 1]
                        nc.scalar.activation(dwslot, ps5[:, D:D + 1], AF.Sigmoid)
                        mdw = colpool.tile([P, 1], f32, tag="mdw")
                        nc.vector.tensor_mul(
                            out=mdw[:], in0=dwslot, in1=mcol[:, jc:jc + 1]
                        )
                        nc.vector.tensor_scalar_mul(
                            Sv[:, 0:2, jc, 0:D],
                            ps12.rearrange("p (k d) -> p k d", k=2),
                            mdw[:],
                        )
                        nc.vector.tensor_scalar_mul(
                            Sv[:, 2:4, jc, 0:D],
                            ps34.rearrange("p (k d) -> p k d", k=2),
                            mdw[:],
                        )
                        nc.vector.tensor_copy(
                            Sv[:, :, jc, D:D + 1],
                            mcolb[:, jc:jc + 1, None].to_broadcast((P, NK, 1)),
                        )
                        nc.vector.tensor_copy(selfsb[:, jc], ps5[:, 0:D])

                    # ---- aggregation matmul + combine ----
                    final = s == STEPS - 1
                    if s == 0:
                        newn = sbpool.tile([P, NJC, D], bf16, tag="newn")
                    else:
                        outst = sbpool.tile([P, NJC, D], f32, tag="outst")
                    for ic in range(NJC):
                        psagg = psC.tile([P, 2 * D], f32, tag="agg")
                        isl = slice(ic * P, (ic + 1) * P)
                        for t in range(TCH):
                            nc.tensor.matmul(
                                psagg[:, :D + 1], lhsT=gt[:, t, isl], rhs=St[:, t],
                                start=(t == 0), stop=(t == TCH - 1),
                            )
                        if s == 0:
                            # nn_num & output row mask, fixed across steps:
                            # fin = mask_i / max(mask_i * rowsum, 1)
                            t1 = colpool.tile([P, 1], f32, tag="t1")
                            nc.vector.tensor_mul(
                                out=t1[:], in0=psagg[:, D:D + 1], in1=mcol[:, ic:ic + 1]
                            )
                            nc.vector.tensor_scalar_max(t1[:], t1[:], 1.0)
                            t2 = colpool.tile([P, 1], f32, tag="t2")
                            nc.vector.reciprocal(t2[:], t1[:])
                            nc.vector.tensor_mul(
                                out=fincol[:, ic:ic + 1], in0=t2[:], in1=mcol[:, ic:ic + 1]
                            )
                        tmp = cmbpool.tile([P, D], f32, tag="cmb")
                        nc.vector.tensor_scalar_mul(
                            tmp[:], psagg[:, 0:D], fincol[:, ic:ic + 1]
                        )
                        nc.vector.tensor_add(out=tmp[:], in0=tmp[:], in1=selfsb[:, ic])
                        if final:
                            nc.scalar.activation(outst[:, ic], tmp[:], AF.Relu)
                        else:
                            nc.scalar.activation(newn[:, ic], tmp[:], AF.Relu)

                    if not final:
                        # transpose new node for the next step's lhsT
                        nt = ntpool.tile([P, KC, N], bf16, tag="nt")
                        for ic in range(NJC):
                            for e in range(KC):
                                pst = psC.tile([P, 4 * D], bf16, tag="agg")
                                nc.tensor.transpose(
                                    pst[:, :P], newn[:, ic, e * P:(e + 1) * P], identt[:]
                                )
                                nc.scalar.copy(
                                    nt[:, e, ic * P:(ic + 1) * P], pst[:, :P]
                                )

                nc.sync.dma_start(
                    out_node[b].rearrange("(c p) d -> p c d", p=P), outst[:]
                )
                with nc.allow_non_contiguous_dma(reason="4KB d_w column store"):
                    nc.sync.dma_start(
                        out_w[b].rearrange("s (c p) -> p s c", p=P),
                        dwt.rearrange("p (s c) -> p s c", s=STEPS),
                    )

    nc.finalize()
    return nc


_NC = None


def _get_nc():
    global _NC
    if _NC is None:
        _NC = build_nc()
    return _NC


def _prep_in_maps(node, node_mask, adj_argument, adj_argument_re, adj_noedges,
                  adj_noedges_re, w_nw, b_nw, w_self, b_self, w_arg, w_arg_prime,
                  w_noe, w_noe_prime):
    adj = np.stack(
        [adj_argument, adj_argument_re, adj_noedges, adj_noedges_re], axis=1
    )  # [B, 4, i, j]
    adjT = np.ascontiguousarray(adj.transpose(0, 1, 3, 2)).astype(BF16)  # [B,4,j,i]
    nodeT = np.ascontiguousarray(
        np.asarray(node, dtype=np.float32).transpose(0, 2, 1)
    ).astype(BF16)  # [B, D, N]
    maskf = np.asarray(node_mask).astype(np.float32)

    w12 = np.concatenate([w_arg.T, w_arg_prime.T], axis=1).astype(BF16)
    w34 = np.concatenate([w_noe.T, w_noe_prime.T], axis=1).astype(BF16)
    w5 = np.concatenate([w_self.T, w_nw.T], axis=1).astype(BF16)
    brow = np.concatenate([b_self, b_nw])[None, :].astype(BF16)
    ident = np.eye(P, dtype=BF16)

    shared = {"w12": w12, "w34": w34, "w5": w5, "brow": brow, "ident": ident}
    in_maps = []
    for i in range(NCORES):
        sl = slice(i * BL, (i + 1) * BL)
        in_maps.append(
            {"adjT": adjT[sl], "nodeT": nodeT[sl], "maskf": maskf[sl], **shared}
        )
    return in_maps


def run(inputs, trace=False, trace_kwargs=None):
    nc = _get_nc()
    in_maps = _prep_in_maps(**inputs)
    res = run_bass_kernel_spmd(
        nc, in_maps, list(range(NCORES)), trace=trace, **(trace_kwargs or {})
    )
    node_out = np.concatenate([res.results[i]["out_node"] for i in range(NCORES)], 0)
    w_out = np.concatenate([res.results[i]["out_w"] for i in range(NCORES)], 0)
    return (node_out, w_out), res


def kernel(**inputs):
    (node_out, w_out), _ = run(inputs)
    return node_out, w_out
